# revision 1
# baseline (speedup 1.0000x reference)
"""Trainium2 Bass kernel for nn_MoELayer (top-1 MoE, dense-masked reference).

Strategy
--------
The reference runs every expert's MLP over every token and then keeps only
the output of each token's argmax-gated expert.  Mathematically the output
for token t is exactly `mlp_{top1(t)}(x_t)`, so we:

  1. compute the (tiny) gate + argmax on the host in float64,
  2. group tokens by chosen expert (expert-parallel sharding: core e gets
     expert e's weights and the tokens routed to it, padded to a fixed
     capacity C),
  3. run one dense MLP per core on its token batch:
        yT = W2^T @ relu(W1^T @ xT + b1) + b2      (all operands natural
     layout: both matmuls produce transposed outputs so no on-device
     transposes are needed; host supplies x pre-transposed),
  4. scatter the per-expert outputs back into the full [B,T,D] tensor.

This does 1/E of the reference FLOPs.  Matmuls use float32r (fp32 operands
truncated to fp22 inside the PE) which streams at full PE rate -- 4x the
true-fp32 matmul rate -- with ~1e-4 relative error, well inside fp32
tolerance for this problem.
"""

import os
import sys

import numpy as np

for _p in ("/opt/trn_rl_repo", "/root/.axon_site/_ro/trn_rl_repo"):
    if os.path.isdir(_p) and _p not in sys.path:
        sys.path.insert(0, _p)

import concourse.bass as bass
import concourse.bacc as bacc
import concourse.mybir as mybir
from concourse.bass_utils import run_bass_kernel_spmd
from concourse.tile import TileContext

# run_bass_kernel_spmd's trace path (BASS_TRACE=1) imports antenv.axon_hooks,
# which not every container ships; force tracing off when it's absent so a
# stray env var can't crash the run.
try:
    from antenv.axon_hooks import get_axon_ntff_profile_hook  # noqa: F401
except Exception:
    os.environ["BASS_NEVER_TRACE"] = "1"

B, T, D, H, E = 4, 2048, 1024, 4096, 8
BT = B * T
N_CORES = 8
F32 = mybir.dt.float32
F32R = mybir.dt.float32r
AF = mybir.ActivationFunctionType

_PROGRAM_CACHE: dict[int, bass.Bass] = {}
LAST_RESULT = None  # BassKernelResults of the most recent device run (for test.py)


def _token_tiles(C):
    """Split C tokens into matmul moving-dim tiles, each in [256, 512]
    (>=256 keeps fp32r at full PE rate; 512 is the fp32 moving-dim max)."""
    assert C >= 512 and C % 8 == 0
    tiles = []
    t0 = 0
    rem = C
    while rem >= 768:
        tiles.append((t0, 512))
        t0 += 512
        rem -= 512
    if rem > 512:
        tiles.append((t0, rem - 256))
        t0 += rem - 256
        rem = 256
    tiles.append((t0, rem))
    return tiles


def _build_program(C: int, repeats: int = 1) -> bass.Bass:
    """One expert MLP over C tokens: yT[D,C] = W2^T @ relu(W1^T @ xT + b1) + b2.

    All 8 cores run this same program on different data (SPMD).

    `repeats` re-runs the whole (idempotent) compute body that many times
    inside one NEFF — used only by test.py to amplify kernel time above the
    axon per-execution launch overhead when measuring.
    """
    nc = bacc.Bacc("TRN2", target_bir_lowering=False, debug=False)

    xT = nc.dram_tensor("xT", [D, C], F32R, kind="ExternalInput").ap()
    w1 = nc.dram_tensor("w1", [D, H], F32R, kind="ExternalInput").ap()
    b1c = nc.dram_tensor("b1c", [128, H // 128], F32, kind="ExternalInput").ap()
    w2 = nc.dram_tensor("w2", [H, D], F32R, kind="ExternalInput").ap()
    b2c = nc.dram_tensor("b2c", [128, D // 128], F32, kind="ExternalInput").ap()
    yT = nc.dram_tensor("yT", [D, C], F32, kind="ExternalOutput").ap()

    ND = D // 128     # 8 chunks of the d (contraction-1 / output-2) axis
    NHC = 8           # h chunks streamed through SBUF
    HC = H // NHC     # 512 hidden units per chunk
    NHS = HC // 128   # 128-row subtiles per h chunk
    NHT = H // 128    # total h tiles
    ttiles = _token_tiles(C)

    with TileContext(nc) as tc:
        with (
            tc.tile_pool(name="const", bufs=1) as constp,
            tc.tile_pool(name="xy", bufs=1) as xyp,
            tc.tile_pool(name="w1p", bufs=2) as w1p,
            tc.tile_pool(name="w2p", bufs=2) as w2p,
            tc.tile_pool(name="htp", bufs=2) as htp,
            tc.tile_pool(name="psA", bufs=4, space="PSUM") as psA,
            tc.tile_pool(name="psB", bufs=4, space="PSUM") as psB,
        ):
            # Warmup ACT with no cross-engine waits: walrus's lower_act
            # attaches the one-time activation-table load to the first ACT
            # instruction, consuming one of its two sync-wait slots. Give it
            # a dependency-free instruction so real ACTs keep both slots.
            warm = constp.tile([128, 1], F32, tag="warm")
            nc.scalar.memzero(warm[:, :])
            nc.scalar.activation(warm[:, :], warm[:, :], AF.Relu)
            nc.scalar.activation(warm[:, :], warm[:, :], AF.Identity)

            # Head-critical DMAs first: the first matmul needs W1-chunk-0
            # and the x blocks, so interleave those ahead of everything
            # else (the tiny bias loads would otherwise occupy the first
            # HWDGE queue slots). x^T resident: partition = d within chunk,
            # col block dc holds xT[dc*128:(dc+1)*128, :].
            xt = xyp.tile([128, ND * C], F32R, tag="xt")
            w1t0 = w1p.tile([128, ND * HC], F32R, tag="w1c")
            for dc in range(ND):
                nc.sync.dma_start(
                    w1t0[:, dc * HC : (dc + 1) * HC],
                    w1[dc * 128 : (dc + 1) * 128, 0:HC],
                )
                nc.sync.dma_start(
                    xt[:, dc * C : (dc + 1) * C],
                    xT[dc * 128 : (dc + 1) * 128, :],
                )

            b1t = constp.tile([128, NHT], F32, tag="b1t")
            nc.sync.dma_start(b1t[:, :], b1c)
            b2t = constp.tile([128, ND], F32, tag="b2t")
            nc.sync.dma_start(b2t[:, :], b2c)

            # y^T accumulator resident, same block layout as xt
            yt = xyp.tile([128, ND * C], F32, tag="yt")

            first_pass = True
            for hc in [h for _ in range(repeats) for h in range(NHC)]:
                # W1 chunk: col block dc holds W1[dc*128:(dc+1)*128, hc*HC:(hc+1)*HC]
                if hc == 0 and first_pass:
                    w1t = w1t0
                    first_pass = False
                else:
                    w1t = w1p.tile([128, ND * HC], F32R, tag="w1c")
                    for dc in range(ND):
                        nc.sync.dma_start(
                            w1t[:, dc * HC : (dc + 1) * HC],
                            w1[dc * 128 : (dc + 1) * 128, hc * HC : (hc + 1) * HC],
                        )
                # W2 chunk: col block hs holds W2[hc*HC+hs*128 : +128, :]
                w2t = w2p.tile([128, NHS * D], F32R, tag="w2c")
                for hs in range(NHS):
                    r0 = hc * HC + hs * 128
                    nc.sync.dma_start(
                        w2t[:, hs * D : (hs + 1) * D], w2[r0 : r0 + 128, :]
                    )

                # h^T chunk: col block hs holds relu(...)[hc*HC+hs*128 : +128, :]
                ht = htp.tile([128, NHS * C], F32R, tag="ht")

                # Phase A: ht = relu(W1c^T @ x + b1c)
                for hs in range(NHS):
                    g = hc * NHS + hs  # global 128-row h tile index
                    for t0, tn in ttiles:
                        ps = psA.tile([128, 512], F32, tag="psA")
                        for dc in range(ND):
                            nc.tensor.matmul(
                                ps[:, :tn],
                                w1t[:, dc * HC + hs * 128 : dc * HC + hs * 128 + 128],
                                xt[:, dc * C + t0 : dc * C + t0 + tn],
                                start=(dc == 0),
                                stop=(dc == ND - 1),
                            )
                        nc.scalar.activation(
                            ht[:, hs * C + t0 : hs * C + t0 + tn],
                            ps[:, :tn],
                            AF.Relu,
                            bias=b1t[:, g : g + 1],
                        )

                # Phase B: yt (+)= W2c^T @ ht   (+ b2 on the first chunk)
                for dt in range(ND):
                    for t0, tn in ttiles:
                        ps = psB.tile([128, 512], F32, tag="psB")
                        for hs in range(NHS):
                            nc.tensor.matmul(
                                ps[:, :tn],
                                w2t[:, hs * D + dt * 128 : hs * D + dt * 128 + 128],
                                ht[:, hs * C + t0 : hs * C + t0 + tn],
                                start=(hs == 0),
                                stop=(hs == NHS - 1),
                            )
                        dst = yt[:, dt * C + t0 : dt * C + t0 + tn]
                        if hc == 0:
                            nc.scalar.activation(
                                dst, ps[:, :tn], AF.Identity, bias=b2t[:, dt : dt + 1]
                            )
                        else:
                            nc.vector.tensor_add(dst, dst, ps[:, :tn])

            for dt in range(ND):
                nc.sync.dma_start(
                    yT[dt * 128 : (dt + 1) * 128, :],
                    yt[:, dt * C : (dt + 1) * C],
                )

    nc.compile()
    return nc


def _get_program(C: int) -> bass.Bass:
    if C not in _PROGRAM_CACHE:
        _PROGRAM_CACHE[C] = _build_program(C)
    return _PROGRAM_CACHE[C]


def _prepare(x, Wg, bg, W1, b1, W2, b2):
    """Host routing: fp64 gate + argmax, group tokens by expert, build the
    per-core (per-expert) input maps padded to capacity C."""
    xf = np.ascontiguousarray(np.asarray(x, dtype=np.float32).reshape(BT, D))

    # Host gate in float64: scores are tiny (BT x E) and fp64 argmax is
    # robust to any fp32 accumulation-order noise in the reference.
    scores = xf.astype(np.float64) @ np.asarray(Wg, dtype=np.float64)
    scores += np.asarray(bg, dtype=np.float64)
    top1 = np.argmax(scores, axis=-1)

    counts = np.bincount(top1, minlength=E)
    # Any C works as long as every token tile has >=256 columns (full
    # fp32r PE rate); keep it 8-aligned (32B DMA lines). Above 1344
    # the resident x/y/h tiles exceed the 192KB/partition SBUF budget,
    # so larger routing skews fall back to multiple passes (never hit
    # for the ~1k-per-expert counts this gate produces).
    C = max(512, int(np.ceil(counts.max() / 8)) * 8)
    C = min(C, 1344)

    W1f = np.asarray(W1, dtype=np.float32)
    b1f = np.asarray(b1, dtype=np.float32)
    W2f = np.asarray(W2, dtype=np.float32)
    b2f = np.asarray(b2, dtype=np.float32)

    in_maps = []
    idxs = []
    for e in range(E):
        idx = np.nonzero(top1 == e)[0]
        idxs.append(idx)
        xe = np.zeros((C, D), dtype=np.float32)
        xe[: min(len(idx), C)] = xf[idx[:C]]
        in_maps.append(
            {
                "xT": np.ascontiguousarray(xe.T),
                "w1": np.ascontiguousarray(W1f[e]),
                "b1c": np.ascontiguousarray(b1f[e].reshape(H // 128, 128).T),
                "w2": np.ascontiguousarray(W2f[e]),
                "b2c": np.ascontiguousarray(b2f[e].reshape(D // 128, 128).T),
            }
        )
    return C, in_maps, idxs


_FASTPATH_CACHE: dict[int, object] = {}


def _make_fastpath(nc):
    """Memoized version of run_bass_kernel_spmd's axon execution path: the
    same sharded custom-call jit, kept alive so repeat kernel() calls skip
    jax retracing and NEFF reload. Numerically identical machinery."""
    import jax
    from jax.sharding import Mesh, PartitionSpec
    from jax.experimental.shard_map import shard_map
    from concourse.bass2jax import (
        _bass_exec_p,
        install_neuronx_cc_hook,
        partition_id_tensor,
    )

    install_neuronx_cc_hook()
    partition_name = nc.partition_id_tensor.name if nc.partition_id_tensor else None
    in_names, out_names, out_avals = [], [], []
    for alloc in nc.m.functions[0].allocations:
        if not isinstance(alloc, mybir.MemoryLocationSet):
            continue
        name = alloc.memorylocations[0].name
        if alloc.kind == "ExternalInput":
            if name != partition_name:
                in_names.append(name)
        elif alloc.kind == "ExternalOutput":
            out_names.append(name)
            out_avals.append(
                jax.core.ShapedArray(tuple(alloc.tensor_shape), mybir.dt.np(alloc.dtype))
            )
    all_names = in_names + out_names + ([partition_name] if partition_name else [])

    def _body(*args):
        operands = list(args)
        if partition_name is not None:
            operands.append(partition_id_tensor())
        return tuple(
            _bass_exec_p.bind(
                *operands,
                out_avals=tuple(out_avals),
                in_names=tuple(all_names),
                out_names=tuple(out_names),
                lowering_input_output_aliases=(),
                sim_require_finite=True,
                sim_require_nnan=True,
                nc=nc,
            )
        )

    mesh = Mesh(np.asarray(jax.devices()[:N_CORES]), ("core",))
    nin, nout = len(in_names), len(out_names)
    fn = jax.jit(
        shard_map(
            _body,
            mesh=mesh,
            in_specs=(PartitionSpec("core"),) * (nin + nout),
            out_specs=(PartitionSpec("core"),) * nout,
            check_rep=False,
        )
    )

    def run(in_maps):
        args = [
            np.concatenate([np.asarray(m[nm]) for m in in_maps], axis=0)
            for nm in in_names
        ]
        for aval in out_avals:
            args.append(np.zeros((N_CORES * aval.shape[0], *aval.shape[1:]), aval.dtype))
        outs = fn(*args)
        return [
            {
                nm: np.asarray(outs[i]).reshape(N_CORES, *out_avals[i].shape)[c]
                for i, nm in enumerate(out_names)
            }
            for c in range(N_CORES)
        ]

    return run


def _run_spmd(C, nc, in_maps):
    global LAST_RESULT
    if C in _FASTPATH_CACHE:
        return _FASTPATH_CACHE[C](in_maps)
    # First call per capacity: the prescribed run_bass_kernel_spmd path
    # (compiles the NEFF); then build the memoized executable for repeats.
    res = run_bass_kernel_spmd(nc, in_maps, list(range(N_CORES)))
    LAST_RESULT = res
    try:
        _FASTPATH_CACHE[C] = _make_fastpath(nc)
    except Exception:
        pass
    return res.results


def kernel(x, Wg, bg, W1, b1, W2, b2):
    C, in_maps, idxs = _prepare(x, Wg, bg, W1, b1, W2, b2)
    nc = _get_program(C)
    results = _run_spmd(C, nc, in_maps)

    out = np.empty((BT, D), dtype=np.float32)
    for e in range(E):
        n_e = min(len(idxs[e]), C)
        if n_e:
            out[idxs[e][:n_e]] = results[e]["yT"][:, :n_e].T

    # Overflow passes: only if some expert drew more than C (=1344) tokens,
    # which this gate's near-uniform routing never does for the given data.
    max_count = max(len(i) for i in idxs)
    done = C
    while done < max_count:
        xf = np.ascontiguousarray(np.asarray(x, dtype=np.float32).reshape(BT, D))
        for e in range(E):
            idx = idxs[e][done : done + C]
            xe = np.zeros((C, D), dtype=np.float32)
            xe[: len(idx)] = xf[idx]
            in_maps[e]["xT"] = np.ascontiguousarray(xe.T)
        results = _run_spmd(C, nc, in_maps)
        for e in range(E):
            idx = idxs[e][done : done + C]
            if len(idx):
                out[idx] = results[e]["yT"][:, : len(idx)].T
        done += C

    return out.reshape(B, T, D)



# revision 2
# speedup vs baseline: 4.6364x; 4.6364x over previous
"""Trainium2 Bass kernel for nn_MoELayer (top-1 MoE, dense-masked reference).

Strategy
--------
The reference runs every expert's MLP over every token and keeps only the
output of each token's argmax-gated expert; so the output for token t is
exactly `mlp_{top1(t)}(x_t)`.  We:

  1. compute the (tiny) gate + argmax on the host in float64,
  2. group tokens by chosen expert (expert-parallel: core e gets expert e's
     weights and its routed tokens, padded to capacity C),
  3. run one dense MLP per core, all matmul operands in bf16:
        yT[D,C] = W2^T @ relu(W1^T @ xT + b1) + b2
     Phase A streams W1 in 8 chunks and writes h (bf16) for ALL 32 h-tiles
     into SBUF; phase B accumulates the full H=4096 contraction (32 matmuls)
     in a single PSUM bank per output tile, so there are no partial-sum
     read-modify-writes.  W2 (bf16, 64KB/partition) is prefetched during
     phase A and fully resident, so phase B has no DMA on its critical path.
  4. scatter the per-expert outputs back into the full [B,T,D] tensor.

bf16 operands halve DMA/SBUF traffic vs fp32r at the same PE rate
(1 col/cycle) and enable fast weight loads; fp32 PSUM accumulation plus an
fp32 output keep rel_l2 error ~3e-3, well inside the 2e-2 gate.  Per-core
PE work is 2 * 256 * C cycles (~234 us at C=1096), which this schedule
approaches on hardware.
"""

import os
import sys

import numpy as np
import ml_dtypes

for _p in ("/opt/trn_rl_repo", "/root/.axon_site/_ro/trn_rl_repo"):
    if os.path.isdir(_p) and _p not in sys.path:
        sys.path.insert(0, _p)

import concourse.bass as bass
import concourse.bacc as bacc
import concourse.mybir as mybir
from concourse.bass_utils import run_bass_kernel_spmd
from concourse.tile import TileContext

# run_bass_kernel_spmd's trace path (BASS_TRACE=1) imports antenv.axon_hooks,
# which not every container ships; force tracing off when it's absent so a
# stray env var can't crash the run.
try:
    from antenv.axon_hooks import get_axon_ntff_profile_hook  # noqa: F401
except Exception:
    os.environ["BASS_NEVER_TRACE"] = "1"

B, T, D, H, E = 4, 2048, 1024, 4096, 8
BT = B * T
N_CORES = 8
F32 = mybir.dt.float32
BF16 = mybir.dt.bfloat16
AF = mybir.ActivationFunctionType
BF16NP = ml_dtypes.bfloat16

ND = D // 128   # 8 d-chunks (contraction blocks of matmul 1 / output tiles of matmul 2)
NHT = H // 128  # 32 h tiles
NHC = 8         # W1 streamed in 8 chunks
HC = H // NHC   # 512 h per chunk

_PROGRAM_CACHE: dict[int, bass.Bass] = {}
LAST_RESULT = None  # BassKernelResults of the most recent device run (for test.py)


def _token_tiles(C):
    """Split C tokens into matmul moving-dim tiles, each in [256, 512]
    (<=512 keeps one fp32 PSUM bank per output tile; >=256 keeps per-
    instruction overheads small)."""
    assert C >= 512 and C % 8 == 0
    tiles = []
    t0 = 0
    rem = C
    while rem >= 768:
        tiles.append((t0, 512))
        t0 += 512
        rem -= 512
    if rem > 512:
        tiles.append((t0, rem - 256))
        t0 += rem - 256
        rem = 256
    tiles.append((t0, rem))
    return tiles


def _build_program(C: int, repeats: int = 1) -> bass.Bass:
    """One expert MLP over C tokens, bf16 operands, fp32 PSUM/output.

    All 8 cores run this same program on different data (SPMD).

    `repeats` re-runs the whole (idempotent) compute body that many times
    inside one NEFF — used only by test.py to amplify kernel time above the
    axon per-execution launch overhead when measuring.
    """
    nc = bacc.Bacc("TRN2", target_bir_lowering=False, debug=False)

    xt_d = nc.dram_tensor("xtp", [128, ND * C], BF16, kind="ExternalInput").ap()
    w1_d = nc.dram_tensor("w1p", [128, NHC * ND * HC], BF16, kind="ExternalInput").ap()
    b1_d = nc.dram_tensor("b1c", [128, NHT], F32, kind="ExternalInput").ap()
    w2_d = nc.dram_tensor("w2p", [128, NHT * D], BF16, kind="ExternalInput").ap()
    b2_d = nc.dram_tensor("b2c", [128, ND], F32, kind="ExternalInput").ap()
    yT = nc.dram_tensor("yT", [D, C], F32, kind="ExternalOutput").ap()

    ttiles = _token_tiles(C)

    with TileContext(nc) as tc:
        with (
            tc.tile_pool(name="const", bufs=1) as constp,
            tc.tile_pool(name="xh", bufs=1) as xhp,
            tc.tile_pool(name="w1pool", bufs=2) as w1pool,
            tc.tile_pool(name="w2pool", bufs=1) as w2pool,
            tc.tile_pool(name="ystp", bufs=4) as ystp,
            tc.tile_pool(name="psA", bufs=4, space="PSUM") as psA,
            tc.tile_pool(name="psB", bufs=4, space="PSUM") as psB,
        ):
            # ACT warmup: detach the one-time activation-table load from real
            # ACTs so they keep both sync-wait slots.
            warm = constp.tile([128, 1], F32, tag="warm")
            nc.scalar.memzero(warm[:, :])
            nc.scalar.activation(warm[:, :], warm[:, :], AF.Relu)
            nc.scalar.activation(warm[:, :], warm[:, :], AF.Identity)

            # Head-critical loads: x then W1 chunk 0 (everything phase A's
            # first matmul group needs).
            xt = xhp.tile([128, ND * C], BF16, tag="xt")
            nc.sync.dma_start(xt[:, :], xt_d)
            w1t0 = w1pool.tile([128, ND * HC], BF16, tag="w1c")
            nc.sync.dma_start(w1t0[:, :], w1_d[:, 0 : ND * HC])

            b1t = constp.tile([128, NHT], F32, tag="b1t")
            nc.sync.dma_start(b1t[:, :], b1_d)
            b2t = constp.tile([128, ND], F32, tag="b2t")
            nc.sync.dma_start(b2t[:, :], b2_d)

            ht = xhp.tile([128, NHT * C], BF16, tag="ht")
            w2t = w2pool.tile([128, NHT * D], BF16, tag="w2t")

            for rep in range(repeats):
                # ---- Phase A: ht[g] = relu(W1[g]^T @ x + b1[g]) ----
                for hc in range(NHC):
                    if hc == 0 and rep == 0:
                        w1t = w1t0
                    else:
                        w1t = w1pool.tile([128, ND * HC], BF16, tag="w1c")
                        nc.sync.dma_start(
                            w1t[:, :], w1_d[:, hc * ND * HC : (hc + 1) * ND * HC]
                        )
                    if hc == 0:
                        # W2 prefetch: has all of phase A to complete.
                        for q in range(8):
                            nc.sync.dma_start(
                                w2t[:, q * 4 * D : (q + 1) * 4 * D],
                                w2_d[:, q * 4 * D : (q + 1) * 4 * D],
                            )
                    for hs in range(4):
                        g = hc * 4 + hs
                        for t0, tn in ttiles:
                            ps = psA.tile([128, 512], F32, tag="psA")
                            for dc in range(ND):
                                nc.tensor.matmul(
                                    ps[:, :tn],
                                    w1t[:, dc * HC + hs * 128 : dc * HC + hs * 128 + 128],
                                    xt[:, dc * C + t0 : dc * C + t0 + tn],
                                    start=(dc == 0),
                                    stop=(dc == ND - 1),
                                )
                            nc.scalar.activation(
                                ht[:, g * C + t0 : g * C + t0 + tn],
                                ps[:, :tn],
                                AF.Relu,
                                bias=b1t[:, g : g + 1],
                            )

                # ---- Phase B: y[dt] = W2^T @ ht + b2 (full-H PSUM accumulation) ----
                for dt in range(ND):
                    for t0, tn in ttiles:
                        ps = psB.tile([128, 512], F32, tag="psB")
                        for hs in range(NHT):
                            nc.tensor.matmul(
                                ps[:, :tn],
                                w2t[:, hs * D + dt * 128 : hs * D + dt * 128 + 128],
                                ht[:, hs * C + t0 : hs * C + t0 + tn],
                                start=(hs == 0),
                                stop=(hs == NHT - 1),
                            )
                        yt = ystp.tile([128, 512], F32, tag="yst")
                        nc.scalar.activation(
                            yt[:, :tn], ps[:, :tn], AF.Identity,
                            bias=b2t[:, dt : dt + 1],
                        )
                        nc.sync.dma_start(
                            yT[dt * 128 : (dt + 1) * 128, t0 : t0 + tn],
                            yt[:, :tn],
                        )

    nc.compile()
    return nc


def _get_program(C: int) -> bass.Bass:
    if C not in _PROGRAM_CACHE:
        _PROGRAM_CACHE[C] = _build_program(C)
    return _PROGRAM_CACHE[C]


def _pack_x(xe):
    """[C, D] fp32 tokens -> bf16 SBUF-block layout [128, ND*C] where
    block dc holds x^T[dc*128:(dc+1)*128, :]."""
    Ctok = xe.shape[0]
    return np.ascontiguousarray(
        xe.reshape(Ctok, ND, 128).transpose(2, 1, 0).reshape(128, ND * Ctok)
    ).astype(BF16NP)


def _prepare(x, Wg, bg, W1, b1, W2, b2):
    """Host routing (fp64 gate + argmax) and bf16 packing into the SBUF
    block layouts the program DMAs directly (each transfer is one
    contiguous [128, N] block)."""
    xf = np.ascontiguousarray(np.asarray(x, dtype=np.float32).reshape(BT, D))

    # Host gate in float64: scores are tiny (BT x E) and fp64 argmax is
    # robust to any fp32 accumulation-order noise in the reference.
    scores = xf.astype(np.float64) @ np.asarray(Wg, dtype=np.float64)
    scores += np.asarray(bg, dtype=np.float64)
    top1 = np.argmax(scores, axis=-1)

    counts = np.bincount(top1, minlength=E)
    # Any C >= 512 multiple of 8 works; above ~1400 the resident x/h/W2
    # tiles would exceed the ~208KB/partition SBUF budget, so extreme
    # routing skew falls back to multiple passes (never hit for the
    # ~1k-per-expert counts this gate produces).
    C = max(512, int(np.ceil(counts.max() / 8)) * 8)
    C = min(C, 1344)

    W1f = np.asarray(W1, dtype=np.float32)
    b1f = np.asarray(b1, dtype=np.float32)
    W2f = np.asarray(W2, dtype=np.float32)
    b2f = np.asarray(b2, dtype=np.float32)

    in_maps = []
    idxs = []
    for e in range(E):
        idx = np.nonzero(top1 == e)[0]
        idxs.append(idx)
        xe = np.zeros((C, D), dtype=np.float32)
        xe[: min(len(idx), C)] = xf[idx[:C]]
        # w1 chunk hc, block dc holds W1[dc*128:(dc+1)*128, hc*HC:(hc+1)*HC]
        w1p = np.ascontiguousarray(
            W1f[e].reshape(ND, 128, NHC, HC).transpose(1, 2, 0, 3).reshape(128, -1)
        ).astype(BF16NP)
        # w2 block hs holds W2[hs*128:(hs+1)*128, :]
        w2p = np.ascontiguousarray(
            W2f[e].reshape(NHT, 128, D).transpose(1, 0, 2).reshape(128, -1)
        ).astype(BF16NP)
        in_maps.append(
            {
                "xtp": _pack_x(xe),
                "w1p": w1p,
                "b1c": np.ascontiguousarray(b1f[e].reshape(NHT, 128).T),
                "w2p": w2p,
                "b2c": np.ascontiguousarray(b2f[e].reshape(ND, 128).T),
            }
        )
    return C, in_maps, idxs


_FASTPATH_CACHE: dict[int, object] = {}


def _make_fastpath(nc):
    """Memoized version of run_bass_kernel_spmd's axon execution path: the
    same sharded custom-call jit, kept alive so repeat kernel() calls skip
    jax retracing and NEFF reload. Numerically identical machinery."""
    import jax
    from jax.sharding import Mesh, PartitionSpec
    from jax.experimental.shard_map import shard_map
    from concourse.bass2jax import (
        _bass_exec_p,
        install_neuronx_cc_hook,
        partition_id_tensor,
    )

    install_neuronx_cc_hook()
    partition_name = nc.partition_id_tensor.name if nc.partition_id_tensor else None
    in_names, out_names, out_avals = [], [], []
    for alloc in nc.m.functions[0].allocations:
        if not isinstance(alloc, mybir.MemoryLocationSet):
            continue
        name = alloc.memorylocations[0].name
        if alloc.kind == "ExternalInput":
            if name != partition_name:
                in_names.append(name)
        elif alloc.kind == "ExternalOutput":
            out_names.append(name)
            out_avals.append(
                jax.core.ShapedArray(tuple(alloc.tensor_shape), mybir.dt.np(alloc.dtype))
            )
    all_names = in_names + out_names + ([partition_name] if partition_name else [])

    def _body(*args):
        operands = list(args)
        if partition_name is not None:
            operands.append(partition_id_tensor())
        return tuple(
            _bass_exec_p.bind(
                *operands,
                out_avals=tuple(out_avals),
                in_names=tuple(all_names),
                out_names=tuple(out_names),
                lowering_input_output_aliases=(),
                sim_require_finite=True,
                sim_require_nnan=True,
                nc=nc,
            )
        )

    mesh = Mesh(np.asarray(jax.devices()[:N_CORES]), ("core",))
    nin, nout = len(in_names), len(out_names)
    fn = jax.jit(
        shard_map(
            _body,
            mesh=mesh,
            in_specs=(PartitionSpec("core"),) * (nin + nout),
            out_specs=(PartitionSpec("core"),) * nout,
            check_rep=False,
        )
    )

    def run(in_maps):
        args = [
            np.concatenate([np.asarray(m[nm]) for m in in_maps], axis=0)
            for nm in in_names
        ]
        for aval in out_avals:
            args.append(np.zeros((N_CORES * aval.shape[0], *aval.shape[1:]), aval.dtype))
        outs = fn(*args)
        return [
            {
                nm: np.asarray(outs[i]).reshape(N_CORES, *out_avals[i].shape)[c]
                for i, nm in enumerate(out_names)
            }
            for c in range(N_CORES)
        ]

    return run


def _run_spmd(C, nc, in_maps):
    global LAST_RESULT
    if C in _FASTPATH_CACHE:
        return _FASTPATH_CACHE[C](in_maps)
    # First call per capacity: the prescribed run_bass_kernel_spmd path
    # (compiles the NEFF); then build the memoized executable for repeats.
    res = run_bass_kernel_spmd(nc, in_maps, list(range(N_CORES)))
    LAST_RESULT = res
    try:
        _FASTPATH_CACHE[C] = _make_fastpath(nc)
    except Exception:
        pass
    return res.results


def kernel(x, Wg, bg, W1, b1, W2, b2):
    C, in_maps, idxs = _prepare(x, Wg, bg, W1, b1, W2, b2)
    nc = _get_program(C)
    results = _run_spmd(C, nc, in_maps)

    out = np.empty((BT, D), dtype=np.float32)
    for e in range(E):
        n_e = min(len(idxs[e]), C)
        if n_e:
            out[idxs[e][:n_e]] = results[e]["yT"][:, :n_e].T

    # Overflow passes: only if some expert drew more than C (=1344) tokens,
    # which this gate's near-uniform routing never does for the given data.
    max_count = max(len(i) for i in idxs)
    done = C
    while done < max_count:
        xf = np.ascontiguousarray(np.asarray(x, dtype=np.float32).reshape(BT, D))
        for e in range(E):
            idx = idxs[e][done : done + C]
            xe = np.zeros((C, D), dtype=np.float32)
            xe[: len(idx)] = xf[idx]
            in_maps[e]["xtp"] = _pack_x(xe)
        results = _run_spmd(C, nc, in_maps)
        for e in range(E):
            idx = idxs[e][done : done + C]
            if len(idx):
                out[idx] = results[e]["yT"][:, : len(idx)].T
        done += C

    return out.reshape(B, T, D)
